# revision 11
# baseline (speedup 1.0000x reference)
"""Trainium2 Bass kernel for nn_CNF1D: 1-D continuous normalizing flow.

Reference computation (per sample b, D=1, H=256, RK4 with 4 steps over [0,1]):
    f(t,z):  h1 = tanh(z*W1[0] + t*W1[1] + b1); h2 = tanh(h1@W2 + b2);
             f = h2@W3 + b3
    JVP:     s1 = 1-h1^2;  g2 = (1-h2^2) * ((s1*W1[0])@W2);  df = g2@W3
    (z, div) integrated with RK4; outputs (z_final, div_integral).

Strategy: pure data parallelism over 8 cores (4096 samples each), organized
as 4 PAIRS of 512-sample chunks per core (processed in 2 quads of 2 pairs).
Hidden-major layout ([hidden, batch]); hidden lives on SBUF partitions.

Per-pair state tile U [128, 512] (fp32r):
  p0-5:    c0 [z, k1z..k4z, ones]   p6-10:  c0 [div, k1d..k4d]
  p32-37:  c0 replica of p0-5 (for row-tiled input matmuls)
  p64-74:  c1 (as p0-10)            p96-101: c1 replica
Input layer: 4 concurrent row-tiled K=6 matmuls (row groups 0/32/64/96).
Output layer: col-tiled M<=2 matmuls (f duplicated to 2 partitions so the
k-row gather including the replica is a single strided DMA).
Evacuations: layer tanh via wide ACTs ([128,1024]); g2 = (g2p + C2) * s2 via
one STT per mo-half over the whole pair ([128,1024]); collector copied once
per QUAD ([128,1024]).

dtypes: state + input/combine matmuls fp32r; streams bf16; PSUM fp32.
"""

import sys

for _p in ("/opt/trn_rl_repo",):
    if _p not in sys.path:
        sys.path.insert(0, _p)

import numpy as np
import ml_dtypes

import concourse.mybir as mybir
from concourse import bacc, tile
from concourse.bass_utils import run_bass_kernel_spmd

F32 = mybir.dt.float32
F32R = mybir.dt.float32r
BF16 = mybir.dt.bfloat16
ALU = mybir.AluOpType
TANH = mybir.ActivationFunctionType.Tanh
COPY = mybir.ActivationFunctionType.Copy

N_CORES = 8
B_TOT = 32768
B = B_TOT // N_CORES        # 4096 per core
H = 256                     # hidden
CH = 512                    # chunk (matmul N / psum bank)
NCH = B // CH               # 8 chunks per core
NPAIR = NCH // 2            # 4 pairs
N_STEPS = 4
DT = 1.0 / N_STEPS
N_EVALS = 4 * N_STEPS       # 16
STAGE_OFF = [0.0, DT / 2, DT / 2, DT]
STAGE_C = [0.0, DT / 2, DT / 2, DT]


def _f32r(x):
    """Round to fp32r (11 explicit mantissa bits, RNE)."""
    b = np.ascontiguousarray(np.asarray(x, np.float32)).view(np.uint32)
    r = (b + np.uint32(0x7FF) + ((b >> np.uint32(12)) & np.uint32(1))) & np.uint32(
        0xFFFFF000
    )
    return r.view(np.float32).copy()


def _build_nc():
    nc = bacc.Bacc("TRN2", target_bir_lowering=False, debug=False,
                   num_devices=N_CORES)

    t0u = nc.dram_tensor("t0u", (NPAIR, 128, CH), F32R, kind="ExternalInput")
    # input-layer weights, replicated per row group:
    #  p0-5: m0-half, p32-37: m1-half, p64-69: m0, p96-101: m1
    lin = nc.dram_tensor("lin", (128, N_EVALS * 128), F32R, kind="ExternalInput")
    combzd = nc.dram_tensor("combzd", (128, 3), F32R, kind="ExternalInput")
    w2 = nc.dram_tensor("w2", (128, 512), BF16, kind="ExternalInput")
    w2gn = nc.dram_tensor("w2gn", (128, 512), BF16, kind="ExternalInput")
    # w3t cols: per mo-half: [f, f, df]  (f duplicated for the M=2 output)
    w3 = nc.dram_tensor("w3", (128, 6), BF16, kind="ExternalInput")
    c2 = nc.dram_tensor("c2", (128, 2), F32, kind="ExternalInput")
    b2 = nc.dram_tensor("b2", (128, 2), F32, kind="ExternalInput")

    zf = nc.dram_tensor("zf", (NCH, CH), F32R, kind="ExternalOutput")
    dv = nc.dram_tensor("dv", (NCH, CH), F32R, kind="ExternalOutput")

    with tile.TileContext(nc) as tc:
        with (
            tc.tile_pool(name="const", bufs=1) as cpool,
            tc.tile_pool(name="state", bufs=1) as spool,
            tc.tile_pool(name="work", bufs=5) as wpool,
            tc.tile_pool(name="ppre", bufs=1, space="PSUM") as ppre,
            tc.tile_pool(name="pmain", bufs=2, space="PSUM") as pmain,
            tc.tile_pool(name="pcoll", bufs=1, space="PSUM") as pcoll,
        ):
            lint = cpool.tile([128, N_EVALS * 128], F32R)
            combt = cpool.tile([128, 3], F32R)
            w2t = cpool.tile([128, 512], BF16)
            w2gnt = cpool.tile([128, 512], BF16)
            w3t = cpool.tile([128, 6], BF16)
            c2t = cpool.tile([128, 2], F32)
            b2t = cpool.tile([128, 2], F32)
            nc.sync.dma_start(lint[:], lin[:])
            nc.sync.dma_start(combt[:], combzd[:])
            nc.sync.dma_start(w2t[:], w2[:])
            nc.sync.dma_start(w2gnt[:], w2gn[:])
            nc.sync.dma_start(w3t[:], w3[:])
            nc.sync.dma_start(c2t[:], c2[:])
            nc.sync.dma_start(b2t[:], b2[:])

            U = []
            for p in range(NPAIR):
                u = spool.tile([128, CH], F32R, tag=f"U{p}")
                nc.sync.dma_start(u[:], t0u[p, :, :])
                U.append(u)

            # round-robin DMA queues for gathers
            dmaq = [nc.sync, nc.gpsimd]
            qi = [0]

            def gdma(dst, src):
                dmaq[qi[0] % 2].dma_start(dst, src)
                qi[0] += 1

            for e in range(N_EVALS):
                s = e % 4
                lslice = lint[:, e * 128:(e + 1) * 128]
                for q in range(2):
                    # -------- per-quad processing (pairs 2q, 2q+1) --------
                    pairs = [2 * q, 2 * q + 1]
                    h2b = {}
                    g2b = {}
                    for pp, p in enumerate(pairs):
                        Up = U[p]
                        # ---- layer 1: 4 row-tiled K=6 matmuls ----
                        h1b = wpool.tile([128, 2048], BF16, tag="h1")
                        for c in range(2):  # chunk within pair
                            pre = ppre.tile([128, 1024], F32, tag="pre")
                            for m in range(2):  # m-half of hidden1
                                rg = 64 * c + 32 * m
                                nc.tensor.matmul(
                                    pre[:, m * CH:(m + 1) * CH],
                                    lint[rg:rg + 6, e * 128:(e + 1) * 128],
                                    Up[rg:rg + 6, :],
                                    tile_position=(rg, 0),
                                )
                            nc.scalar.activation(
                                h1b[:, c * 1024:(c + 1) * 1024], pre[:], TANH
                            )
                        sq1 = wpool.tile([128, 2048], BF16, tag="sq1")
                        nc.vector.tensor_tensor(sq1[:], h1b[:], h1b[:], ALU.mult)

                        # ---- layer 2 ----
                        h2 = wpool.tile([128, 2048], BF16, tag="h2")
                        s2 = wpool.tile([128, 2048], BF16, tag="s2")
                        g2 = wpool.tile([128, 2048], BF16, tag="g2")
                        for mo in range(2):
                            mslice = slice(mo * 1024, (mo + 1) * 1024)
                            a2 = pmain.tile([128, 1024], F32, tag="a2")
                            for k in range(2):
                                for c in range(2):
                                    nc.tensor.matmul(
                                        a2[:, c * CH:(c + 1) * CH],
                                        w2t[:, k * 256 + mo * 128:
                                            k * 256 + (mo + 1) * 128],
                                        h1b[:, c * 1024 + k * CH:
                                            c * 1024 + (k + 1) * CH],
                                        start=(k == 0),
                                        stop=(k == 1),
                                    )
                            nc.scalar.activation(
                                h2[:, mslice], a2[:], TANH,
                                bias=b2t[:, mo:mo + 1],
                            )
                            # s2 = 1 - h2^2 (per mo-half, so the g-stream STT
                            # for this half never waits on the other half)
                            sq2 = wpool.tile([128, 1024], BF16, tag="sq2")
                            nc.vector.tensor_tensor(
                                sq2[:], h2[:, mslice], h2[:, mslice], ALU.mult
                            )
                            nc.vector.tensor_scalar(
                                s2[:, mslice], sq2[:], -1.0, 1.0,
                                ALU.mult, ALU.add,
                            )
                            g2p = pmain.tile([128, 1024], F32, tag="a2")
                            for k in range(2):
                                for c in range(2):
                                    nc.tensor.matmul(
                                        g2p[:, c * CH:(c + 1) * CH],
                                        w2gnt[:, k * 256 + mo * 128:
                                              k * 256 + (mo + 1) * 128],
                                        sq1[:, c * 1024 + k * CH:
                                            c * 1024 + (k + 1) * CH],
                                        start=(k == 0),
                                        stop=(k == 1),
                                    )
                            nc.vector.scalar_tensor_tensor(
                                g2[:, mslice], g2p[:],
                                c2t[:, mo:mo + 1],
                                s2[:, mslice],
                                ALU.add, ALU.mult,
                            )
                        h2b[pp] = h2
                        g2b[pp] = g2

                    # ---- output layer for the quad: col-tiled into coll ----
                    coll = pcoll.tile([128, 1024], F32, tag="coll")
                    for pp in range(2):
                        off = pp * CH
                        for mo in range(2):
                            st, sp = (mo == 0), (mo == 1)
                            for c in range(2):
                                # f -> partitions {64c, 64c+1} (M=2 dup)
                                nc.tensor.matmul(
                                    coll[64 * c:64 * c + 2, off:off + CH],
                                    w3t[:, 3 * mo:3 * mo + 2],
                                    h2b[pp][:, mo * 1024 + c * CH:
                                            mo * 1024 + (c + 1) * CH],
                                    start=st, stop=sp,
                                    tile_position=(0, 64 * c),
                                )
                                # df -> partition {64c+32}
                                nc.tensor.matmul(
                                    coll[64 * c + 32:64 * c + 33, off:off + CH],
                                    w3t[:, 3 * mo + 2:3 * mo + 3],
                                    g2b[pp][:, mo * 1024 + c * CH:
                                            mo * 1024 + (c + 1) * CH],
                                    start=st, stop=sp,
                                    tile_position=(0, 64 * c + 32),
                                )
                    scr = wpool.tile([128, 1024], F32R, tag="scr")
                    nc.scalar.activation(scr[:], coll[:], COPY)

                    # ---- gathers: k-rows back into U (incl. replicas) ----
                    for pp, p in enumerate(pairs):
                        off = pp * CH
                        Up = U[p]
                        for c in range(2):
                            base = 64 * c
                            # f (2 copies) -> z-block k-row + replica
                            gdma(Up[base + 1 + s:base + 34 + s:32, :],
                                 scr[base:base + 2, off:off + CH])
                            # df -> div-block k-row
                            gdma(Up[base + 7 + s:base + 8 + s, :],
                                 scr[base + 32:base + 33, off:off + CH])

                    if s == 3:
                        # ---- RK4 combine: one K=11 M=3 matmul per chunk.
                        # fp32r blocks col-tiling, so every matmul writes
                        # rows 0-2 (col group 0); chunks get separate
                        # 512-col halves of a per-pair cc tile. ----
                        for pp, p in enumerate(pairs):
                            cc = pcoll.tile([128, 1024], F32, tag="coll")
                            for c in range(2):
                                base = 64 * c
                                nc.tensor.matmul(
                                    cc[0:3, c * CH:(c + 1) * CH],
                                    combt[base:base + 11, :],
                                    U[p][base:base + 11, :],
                                    tile_position=(base, 0),
                                )
                            scr2 = wpool.tile([128, 1024], F32R, tag="scr")
                            nc.scalar.activation(
                                scr2[0:3, :], cc[0:3, :], COPY)
                            for c in range(2):
                                base = 64 * c
                                ccol = c * CH
                                ch = 4 * q + 2 * pp + c
                                if e == N_EVALS - 1:
                                    nc.sync.dma_start(
                                        zf[ch:ch + 1, :],
                                        scr2[0:1, ccol:ccol + CH])
                                    nc.sync.dma_start(
                                        dv[ch:ch + 1, :],
                                        scr2[2:3, ccol:ccol + CH])
                                else:
                                    # z -> rows {0, 32}+base, div -> row 6+base
                                    gdma(U[p][base:base + 33:32, :],
                                         scr2[0:2, ccol:ccol + CH])
                                    gdma(U[p][base + 6:base + 7, :],
                                         scr2[2:3, ccol:ccol + CH])

    nc.compile()
    return nc


_NC_CACHE = None


def _get_nc():
    global _NC_CACHE
    if _NC_CACHE is None:
        _NC_CACHE = _build_nc()
    return _NC_CACHE


def _host_prep(z0, W1, b1, W2, b2, W3, b3):
    """Build per-core input maps (host-side folds; all tiny)."""
    z0 = np.asarray(z0, np.float32)
    W1 = np.asarray(W1, np.float32)
    b1 = np.asarray(b1, np.float32)
    W2 = np.asarray(W2, np.float32)
    b2v = np.asarray(b2, np.float32)
    W3 = np.asarray(W3, np.float32)
    b3v = float(np.asarray(b3, np.float32).reshape(()))

    w1r0, w1r1 = W1[0], W1[1]

    # lin: [128 partitions, N_EVALS*128]; row groups 0/64 carry the m0-half
    # of the input weights, 32/96 the m1-half (6 K-rows each).
    lin = np.zeros((128, N_EVALS * 128), np.float32)
    for e in range(N_EVALS):
        i, s = divmod(e, 4)
        t_e = i * DT + STAGE_OFF[s]
        c_e = STAGE_C[s]
        blk6 = np.zeros((6, H), np.float32)
        blk6[0] = w1r0
        if s >= 1:
            blk6[s] = c_e * w1r0
        blk6[5] = t_e * w1r1 + b1 + c_e * b3v * w1r0
        for rg, m in ((0, 0), (32, 1), (64, 0), (96, 1)):
            lin[rg:rg + 6, e * 128:(e + 1) * 128] = blk6[:, m * 128:(m + 1) * 128]

    combzd = np.zeros((128, 3), np.float32)
    zcol = [1.0, DT / 6, DT / 3, DT / 3, DT / 6, DT * b3v, 0, 0, 0, 0, 0]
    dcol = [0, 0, 0, 0, 0, 0, 1.0, DT / 6, DT / 3, DT / 3, DT / 6]
    for base in (0, 64):
        combzd[base:base + 11, 0] = zcol
        combzd[base:base + 11, 1] = zcol
        combzd[base:base + 11, 2] = dcol

    w2p = np.concatenate([W2[0:128, :], W2[128:256, :]], axis=1)  # [128,512]
    w2g = W2 * w1r0[:, None]
    w2gnp = np.concatenate([-w2g[0:128, :], -w2g[128:256, :]], axis=1)
    c2 = w2g.sum(axis=0)  # [256]
    c2p = np.stack([c2[0:128], c2[128:256]], axis=1)  # [128,2]
    b2p = np.stack([b2v[0:128], b2v[128:256]], axis=1)
    # w3t: per mo-half cols [f, f, df]
    w3p = np.zeros((128, 6), np.float32)
    for mo in range(2):
        col = W3[mo * 128:(mo + 1) * 128, 0]
        w3p[:, 3 * mo] = col
        w3p[:, 3 * mo + 1] = col
        w3p[:, 3 * mo + 2] = col

    shared = {
        "lin": _f32r(lin),
        "combzd": _f32r(combzd),
        "w2": w2p.astype(ml_dtypes.bfloat16),
        "w2gn": w2gnp.astype(ml_dtypes.bfloat16),
        "w3": w3p.astype(ml_dtypes.bfloat16),
        "c2": c2p,
        "b2": b2p,
    }
    in_maps = []
    for core in range(N_CORES):
        zc = z0[core * B:(core + 1) * B, 0].reshape(NCH, CH)
        t0uv = np.zeros((NPAIR, 128, CH), np.float32)
        for p in range(NPAIR):
            for c in range(2):
                base = 64 * c
                zrow = _f32r(zc[2 * p + c])
                t0uv[p, base + 0, :] = zrow
                t0uv[p, base + 5, :] = 1.0
                t0uv[p, base + 32, :] = zrow
                t0uv[p, base + 37, :] = 1.0
        in_maps.append({"t0u": t0uv, **shared})
    return in_maps


def _run(in_maps, **kw):
    nc = _get_nc()
    return run_bass_kernel_spmd(nc, in_maps, core_ids=list(range(N_CORES)), **kw)


def kernel(z0, W1, b1, W2, b2, W3, b3):
    in_maps = _host_prep(z0, W1, b1, W2, b2, W3, b3)
    res = _run(in_maps)
    zf = np.concatenate(
        [np.asarray(r["zf"], np.float32).reshape(B, 1) for r in res.results]
    )
    dv = np.concatenate(
        [np.asarray(r["dv"], np.float32).reshape(B, 1) for r in res.results]
    )
    return zf, dv
